# revision 8
# baseline (speedup 1.0000x reference)
"""Causal self-attention block (nn_CrossAttention) on 8 TRN2 NeuronCores.

Sharding: data-parallel over batch (B=2 -> 2 groups of 4 cores), tensor-parallel
over heads within a group (16 heads -> 4 heads/core, splitting Wq/Wk/Wv rows and
Wp columns). Each core computes a full [N, DIM] partial of the output projection
for its 4 heads; the host sums the 4 partials per batch and adds the bias.

Device-side layout ("transposed world", everything feature-major):
  xT   [C=1024, N=2048]    QT/KT/VT = W @ xT -> [d, n] with d on partitions
  V    = PE-transpose of VT blocks -> [l, d] per 128-block, packed per head
         with a 64-wide ones block ([V_h|ones] even heads, [ones|V_h] odd)
  S^T  = K_j @ Q^T chunks  -> [l, n] in PSUM (l = key block on partitions)
  P^T  = exp(SCALE*S^T) -> SBUF bf16, causal-masked by a 0/1 multiply
  O''  = [V_j|ones].T @ P^T accumulated in PSUM: O rows + row-sum rows
  out  = (O/s).T-pair @ WpT -> [n, e] partial, bf16 to DRAM

No max-subtraction is needed in the softmax (logits*scale max ~8).

Softmax normalization is division-free: 1/s = exp(-ln s) runs on the ACT
engine, whose table set natural_log_exp_and_others serves Exp, Ln and Copy
at once — a single manually-emitted InstLoadActFuncSet(6) up front keeps
the fixpoint table-load pass from thrashing exp<->ln tables.

The O''+rowsum accumulator is four 1-bank PSUM tiles (2 head-parity x 2
512-column halves) in a 4-deep ring; each half is normalized as soon as
its last causal key block lands (j=8q+4h+3), so the normalize latency
overlaps the remaining attention and the ring hands banks to the next
(t,q) block with almost no PE stall. Output-projection row blocks are
emitted as early as their onorm halves exist, overlapping the last
attention phases and spreading the (bf16) output DMA.
"""

import numpy as np
import ml_dtypes

B = 2
N = 2048
DIM = 1024
H = 16
D = 64
SCALE = D ** -0.5
NCORES = 8
HPC = 4          # heads per core
FPC = HPC * D    # feature rows per core (256)

_BF = ml_dtypes.bfloat16

_built = None


def _split512(lo, hi):
    """Split [lo, hi) at multiples of 512 (PSUM bank boundaries)."""
    out = []
    p = lo
    while p < hi:
        q = min(hi, (p // 512 + 1) * 512)
        out.append((p, q))
        p = q
    return out


def _build(loop_k=None):
    """Build the (SPMD, data-only-sharded) Bass program. Same NEFF on all cores.

    loop_k: if set, wrap the whole compute body in a hardware For-loop that
    repeats it loop_k times (used only for timing-by-differencing in bench.py;
    the graded path uses loop_k=None).
    """
    import concourse.bass as bass
    import concourse.mybir as mybir
    import concourse.tile as tile
    from concourse import bacc
    from contextlib import ExitStack

    bf16 = mybir.dt.bfloat16
    f32 = mybir.dt.float32
    Exp = mybir.ActivationFunctionType.Exp
    Ln = mybir.ActivationFunctionType.Ln

    nc = bacc.Bacc()
    # Host pre-lays-out weights so every DMA row is 4KB-contiguous per
    # partition (512B packets from the "(a p) d" rearrange are ~4x slower).
    xT_d = nc.dram_tensor("xT", [128, DIM // 128, N], bf16, kind="ExternalInput")
    wqT_d = nc.dram_tensor("wqT", [128, DIM // 128, FPC], bf16, kind="ExternalInput")
    wkT_d = nc.dram_tensor("wkT", [128, DIM // 128, FPC], bf16, kind="ExternalInput")
    wvT_d = nc.dram_tensor("wvT", [128, DIM // 128, FPC], bf16, kind="ExternalInput")
    wpT_d = nc.dram_tensor("wpT", [128, FPC // 128, DIM], bf16, kind="ExternalInput")
    mask_d = nc.dram_tensor("mask01", [128, 128], bf16, kind="ExternalInput")
    ident_d = nc.dram_tensor("ident", [128, 128], bf16, kind="ExternalInput")
    out_d = nc.dram_tensor("out", [N, DIM], bf16, kind="ExternalOutput")

    NB = N // 128      # 16 blocks of 128 along sequence
    KC = DIM // 128    # 8 contraction chunks

    with tile.TileContext(nc) as tc, ExitStack() as ctx:
        sing = ctx.enter_context(tc.tile_pool(name="sing", bufs=1))
        pspool = ctx.enter_context(tc.tile_pool(name="pspool", bufs=2, space="PSUM"))
        o2pool = ctx.enter_context(tc.tile_pool(name="o2pool", bufs=4, space="PSUM"))
        ptpool = ctx.enter_context(tc.tile_pool(name="ptpool", bufs=4))
        rcpool = ctx.enter_context(tc.tile_pool(name="rcpool", bufs=4))
        outpool = ctx.enter_context(tc.tile_pool(name="outpool", bufs=3))

        if loop_k is not None:
            ctx.enter_context(tc.For_i(
                0, loop_k, 1,
                hint_engines=(
                    mybir.EngineType.PE, mybir.EngineType.Activation,
                    mybir.EngineType.DVE, mybir.EngineType.SP,
                ),
            ))

        # One table set serves every ACT use (Exp, Ln, Copy): preload it so
        # the automatic pass inserts no exp<->ln table swaps (~1.3us each).
        nc.scalar.add_instruction(mybir.InstLoadActFuncSet(
            name=nc.get_next_instruction_name(), ins=[], outs=[],
            act_func_set_id=6,  # natural_log_exp_and_others
        ))

        xTs = sing.tile([128, KC, N], bf16)
        wqTs = sing.tile([128, KC, FPC], bf16)
        wkTs = sing.tile([128, KC, FPC], bf16)
        wvTs = sing.tile([128, KC, FPC], bf16)
        wpTs = sing.tile([128, 2, DIM], bf16)
        qTs = sing.tile([128, 2, N], bf16)
        kTs = sing.tile([128, 2, N], bf16)
        vTs = sing.tile([128, 2, N], bf16)
        # v2: per (l-block j, head h) a contiguous 128-col weight slot:
        # even h -> [V_h | ones], odd h -> [ones | V_h]  (so O lands on
        # partitions [64*(h%2), +64) and the row-sums on the other half)
        v2 = sing.tile([128, NB, HPC, 128], bf16)
        onorm = sing.tile([128, 2, N], bf16)
        maskS = sing.tile([128, 128], bf16)
        identS = sing.tile([128, 128], bf16)

        # ---- input DMAs (first-needed first; wq per-chunk so the first
        # projection matmuls pipeline behind the x chunk arrivals) ----
        nc.sync.dma_start(out=wqTs[:, 0, :], in_=wqT_d[:, 0, :])
        for a in range(KC):
            nc.sync.dma_start(out=xTs[:, a, :], in_=xT_d[:, a, :])
            if a + 1 < KC:
                nc.sync.dma_start(out=wqTs[:, a + 1, :], in_=wqT_d[:, a + 1, :])
        nc.sync.dma_start(out=wkTs, in_=wkT_d[:])
        nc.sync.dma_start(out=wvTs, in_=wvT_d[:])
        nc.sync.dma_start(out=identS, in_=ident_d[:, :])
        nc.sync.dma_start(out=maskS, in_=mask_d[:, :])
        nc.sync.dma_start(out=wpTs, in_=wpT_d[:])

        for h in range(HPC):
            ones_cols = slice(64, 128) if h % 2 == 0 else slice(0, 64)
            nc.vector.memset(v2[:, :, h, ones_cols], 1.0)

        # ---- Q/K/V projections (weight-stationary, transposed outputs) ----
        def proj_block(wt, dst, t):
            for cc in range(2):
                ps = pspool.tile([128, 1024], f32, tag="ps", name=f"qkv_ps")
                for half in range(2):
                    n0 = 1024 * cc + 512 * half
                    for k in range(KC):
                        nc.tensor.matmul(
                            ps[:, 512 * half:512 * half + 512],
                            lhsT=wt[:, k, 128 * t:128 * (t + 1)],
                            rhs=xTs[:, k, n0:n0 + 512],
                            start=(k == 0), stop=(k == KC - 1),
                        )
                nc.vector.tensor_copy(
                    out=dst[:, t, 1024 * cc:1024 * (cc + 1)], in_=ps[:, :]
                )

        def v_transpose_block(t):
            # vTs[:, t, :] rows are d-dims of heads (2t, 2t+1); transpose each
            # 128x128 l-block back to [l, d] and scatter into the two heads'
            # v2 slots ([V|ones] / [ones|V]).
            vj_all = v2[:, :, :, :]
            part_d = list(vj_all.ap)[0]
            for j in range(NB):
                vt_ps = pspool.tile([128, 128], bf16, tag="ps", name="vt_ps")
                nc.tensor.transpose(
                    vt_ps[:, :], vTs[:, t, 128 * j:128 * (j + 1)], identS[:, :]
                )
                # dst: head 2t cols 0:64 and head 2t+1 cols 64:128
                dst = bass.AP(
                    tensor=vj_all.tensor,
                    offset=vj_all.offset + j * HPC * 128 + 256 * t,
                    ap=[[part_d[0], part_d[1]], [192, 2], [1, 64]],
                )
                nc.vector.tensor_copy(out=dst, in_=vt_ps[:, :])

        def norm_bank(t, q, par, h, o2):
            """Normalize o2[(par,h)] (one 512-col half) into onorm."""
            nlo = 1024 * q + 512 * h
            r = 64 * par
            sb = 64 - r
            rc = rcpool.tile([128, 512], f32, tag="rc", name="rc")
            # 1/s = exp(-ln s): two cheap ACT passes, no table swap, and no
            # 6.5us DVE reciprocal on the o2-release path.
            nc.scalar.activation(
                out=rc[sb:sb + 64, :], in_=o2[par, h][sb:sb + 64, :], func=Ln
            )
            nc.scalar.activation(
                out=rc[sb:sb + 64, :], in_=rc[sb:sb + 64, :],
                func=Exp, scale=-1.0,
            )
            # move 1/s onto O's partitions (DMA shuffles partitions)
            nc.sync.dma_start(out=rc[r:r + 64, :], in_=rc[sb:sb + 64, :])
            nc.vector.tensor_mul(
                out=onorm[r:r + 64, t, nlo:nlo + 512],
                in0=o2[par, h][r:r + 64, :], in1=rc[r:r + 64, :],
            )

        # ---- output projection: out[n_blk, e] = sum_pair O_pair.T @ WpT_pair ----
        def proj_out(nb):
            po = pspool.tile([128, 1024], f32, tag="ps", name="po")
            for half in range(2):
                for p in range(2):
                    nc.tensor.matmul(
                        po[:, 512 * half:512 * half + 512],
                        lhsT=onorm[:, p, 128 * nb:128 * (nb + 1)],
                        rhs=wpTs[:, p, 512 * half:512 * half + 512],
                        start=(p == 0), stop=(p == 1),
                    )
            ostage = outpool.tile([128, 1024], bf16, tag="ostage", name="ostage")
            if nb % 2 == 0:
                nc.vector.tensor_copy(out=ostage, in_=po)
            else:
                nc.scalar.copy(out=ostage, in_=po)
            nc.sync.dma_start(out=out_d[128 * nb:128 * (nb + 1), :], in_=ostage)

        # ---- attention for head pair t, query half q (S^T -> exp -> O'') ----
        # o2 is four 1-bank tiles [(par, h)]; each half h is normalized as
        # soon as its last causal key block (j = 8q+4h+3) has accumulated.
        # after_j: {j: [fn]} work to emit right after iteration j (e.g.
        # output-projection blocks that only need already-normalized halves).
        def attn_half(t, q, after_j=None):
            nlo, nhi = 1024 * q, 1024 * (q + 1)
            o2 = {}
            for h in range(2):
                for par in range(2):
                    o2[par, h] = o2pool.tile(
                        [128, 512], f32, tag="o2", name=f"o2_{t}_{q}_{par}_{h}"
                    )
            for j in range(8 * (q + 1)):
                a0 = 128 * j
                lo = max(a0, nlo)
                pieces = _split512(lo, nhi)
                st = {par: pspool.tile([128, 1024], f32, tag="ps", name="st")
                      for par in range(2)}
                # even/odd adjacent per piece: K=64 at base partitions 0/64
                for p0, p1 in pieces:
                    for par in range(2):
                        r = 64 * par
                        nc.tensor.matmul(
                            st[par][:, p0 - nlo:p1 - nlo],
                            lhsT=kTs[r:r + 64, t, a0:a0 + 128],
                            rhs=qTs[r:r + 64, t, p0:p1],
                            start=True, stop=True,
                        )
                pt = {}
                for par in range(2):
                    pt[par] = ptpool.tile([128, 1024], bf16, tag="pt", name="pt")
                    nc.scalar.activation(
                        out=pt[par][:, lo - nlo:nhi - nlo],
                        in_=st[par][:, lo - nlo:nhi - nlo],
                        func=Exp, scale=SCALE,
                    )
                    if lo == a0:  # this chunk starts at the diagonal block
                        nc.vector.tensor_mul(
                            pt[par][:, a0 - nlo:a0 - nlo + 128],
                            pt[par][:, a0 - nlo:a0 - nlo + 128],
                            maskS,
                        )
                for par in range(2):
                    hh = 2 * t + par
                    vap = v2[:, j, hh, :]
                    for p0, p1 in pieces:
                        h = (p0 - nlo) // 512
                        nc.tensor.matmul(
                            o2[par, h][:, p0 - nlo - 512 * h:p1 - nlo - 512 * h],
                            lhsT=vap,
                            rhs=pt[par][:, p0 - nlo:p1 - nlo],
                            start=(j == 0),
                            stop=(j == 8 * q + 4 * h + 3),
                        )
                for h in range(2):
                    if j == 8 * q + 4 * h + 3:
                        for par in range(2):
                            norm_bank(t, q, par, h, o2)
                if after_j and j in after_j:
                    for fn in after_j[j]:
                        fn()

        # ---- schedule ----
        proj_block(wqTs, qTs, 0)
        proj_block(wkTs, kTs, 0)
        proj_block(wvTs, vTs, 0)
        v_transpose_block(0)

        attn_half(0, 0)
        proj_block(wqTs, qTs, 1)          # boundary filler: independent PE work
        attn_half(0, 1)
        proj_block(wkTs, kTs, 1)          # filler
        proj_block(wvTs, vTs, 1)
        v_transpose_block(1)

        # rows 0..511 need only the h=0 halves of (t,q=0) blocks; emit them
        # inside attn(1,0) right after its h0 normalize (j=3). rows 512..1023
        # additionally need h=1 (ready at the end of attn(1,0)).
        attn_half(1, 0, after_j={3: [lambda nb=nb: proj_out(nb) for nb in range(4)]})
        for nb in range(4, 8):
            proj_out(nb)
        attn_half(1, 1, after_j={11: [lambda nb=nb: proj_out(nb) for nb in range(8, 12)]})
        for nb in range(12, NB):
            proj_out(nb)

    nc.finalize()
    return nc


def _get_nc():
    global _built
    if _built is None:
        _built = _build()
    return _built


def _chunk128(a, kc):
    """[kc*128, m] -> [128, kc, m] so each partition row is contiguous."""
    m = a.shape[1]
    return np.ascontiguousarray(
        a.reshape(kc, 128, m).transpose(1, 0, 2)
    )


def make_in_maps(x, Wq, Wk, Wv, Wp):
    # 0 where key>query (strictly-lower in [l, n] coords), else 1
    mask = np.where(
        np.arange(128)[:, None] > np.arange(128)[None, :], 0.0, 1.0
    ).astype(_BF)
    ident = np.eye(128, dtype=np.float32).astype(_BF)
    in_maps = []
    for c in range(NCORES):
        b, g = c // HPC, c % HPC
        rows = slice(FPC * g, FPC * (g + 1))
        in_maps.append({
            "xT": _chunk128(np.ascontiguousarray(x[b].T).astype(_BF), DIM // 128),
            "wqT": _chunk128(np.ascontiguousarray(Wq[rows, :].T).astype(_BF), DIM // 128),
            "wkT": _chunk128(np.ascontiguousarray(Wk[rows, :].T).astype(_BF), DIM // 128),
            "wvT": _chunk128(np.ascontiguousarray(Wv[rows, :].T).astype(_BF), DIM // 128),
            "wpT": _chunk128(np.ascontiguousarray(Wp[:, rows].T).astype(_BF), FPC // 128),
            "mask01": mask,
            "ident": ident,
        })
    return in_maps


def run_sharded(x, Wq, Wk, Wv, Wp, bp, trace=False, **spmd_kwargs):
    from concourse.bass_utils import run_bass_kernel_spmd

    nc = _get_nc()
    in_maps = make_in_maps(x, Wq, Wk, Wv, Wp)
    res = run_bass_kernel_spmd(
        nc, in_maps, core_ids=list(range(NCORES)), trace=trace, **spmd_kwargs
    )
    parts = [r["out"] for r in res.results]
    out = np.zeros((B, N, DIM), np.float32)
    for b in range(B):
        acc = np.zeros((N, DIM), np.float32)
        for g in range(HPC):
            acc += parts[b * HPC + g].astype(np.float32)
        out[b] = acc + bp.astype(np.float32)[None, :]
    return out, res


def kernel(x, y, Wq, Wk, Wv, Wp, bp):
    x = np.asarray(x, np.float32)
    out, _ = run_sharded(
        x,
        np.asarray(Wq, np.float32), np.asarray(Wk, np.float32),
        np.asarray(Wv, np.float32), np.asarray(Wp, np.float32),
        np.asarray(bp, np.float32),
    )
    return out


# revision 9
# speedup vs baseline: 1.2031x; 1.2031x over previous
"""Causal self-attention block (nn_CrossAttention) on 8 TRN2 NeuronCores.

Sharding: data-parallel over batch (B=2 -> 2 groups of 4 cores), tensor-parallel
over heads within a group (16 heads -> 4 heads/core, splitting Wq/Wk/Wv rows and
Wp columns). Each core computes a full [N, DIM] partial of the output projection
for its 4 heads; the host sums the 4 partials per batch and adds the bias.

Device-side layout ("transposed world", everything feature-major):
  xT   [C=1024, N=2048]    QT/KT/VT = W @ xT -> [d, n] with d on partitions
  V    = PE-transpose of VT blocks -> [l, d] per 128-block, packed per head
         with a 64-wide ones block ([V_h|ones] even heads, [ones|V_h] odd)
  S^T  = K_j @ Q^T chunks  -> [l, n] in PSUM (l = key block on partitions)
  P^T  = exp(SCALE*S^T) -> SBUF bf16, causal-masked by a 0/1 multiply
  O''  = [V_j|ones].T @ P^T accumulated in PSUM: O rows + row-sum rows
  out  = (O/s).T-pair @ WpT -> [n, e] partial, bf16 to DRAM

No max-subtraction is needed in the softmax (logits*scale max ~8).

Softmax normalization is division-free and staged so the PSUM accumulator
ring never waits on it: o2 is released by a fast pair (DVE copies the O
rows to SBUF staging || ACT takes ln of the row-sums straight from PSUM);
1/s = exp(-ln s) (ACT; Exp+Ln+Copy share table set 6,
natural_log_exp_and_others, preloaded once so the table-load pass emits
no swaps), a DMA shuffles 1/s onto O's partitions, and DVE multiplies
into onorm -- all off the o2-ring critical path.

Schedule: t=1 QKV/V-transpose blocks fill the (t=0) normalize boundaries;
attention (1,1) starts right after (1,0) with only the staged release in
between; output-projection row blocks 0-7 are injected into attention
(1,1) once onorm rows 0-1023 exist, overlapping the final attention phase
and spreading the (bf16) output DMA.
"""

import numpy as np
import ml_dtypes

B = 2
N = 2048
DIM = 1024
H = 16
D = 64
SCALE = D ** -0.5
NCORES = 8
HPC = 4          # heads per core
FPC = HPC * D    # feature rows per core (256)

_BF = ml_dtypes.bfloat16

_built = None


def _split512(lo, hi):
    """Split [lo, hi) at multiples of 512 (PSUM bank boundaries)."""
    out = []
    p = lo
    while p < hi:
        q = min(hi, (p // 512 + 1) * 512)
        out.append((p, q))
        p = q
    return out


def _build(loop_k=None):
    """Build the (SPMD, data-only-sharded) Bass program. Same NEFF on all cores.

    loop_k: if set, wrap the whole compute body in a hardware For-loop that
    repeats it loop_k times (used only for timing-by-differencing in bench.py;
    the graded path uses loop_k=None).
    """
    import concourse.bass as bass
    import concourse.mybir as mybir
    import concourse.tile as tile
    from concourse import bacc
    from contextlib import ExitStack

    bf16 = mybir.dt.bfloat16
    f32 = mybir.dt.float32
    Exp = mybir.ActivationFunctionType.Exp
    Ln = mybir.ActivationFunctionType.Ln

    nc = bacc.Bacc()
    # Host pre-lays-out weights so every DMA row is 4KB-contiguous per
    # partition (512B packets from the "(a p) d" rearrange are ~4x slower).
    xT_d = nc.dram_tensor("xT", [128, DIM // 128, N], bf16, kind="ExternalInput")
    wqT_d = nc.dram_tensor("wqT", [128, DIM // 128, FPC], bf16, kind="ExternalInput")
    wkT_d = nc.dram_tensor("wkT", [128, DIM // 128, FPC], bf16, kind="ExternalInput")
    wvT_d = nc.dram_tensor("wvT", [128, DIM // 128, FPC], bf16, kind="ExternalInput")
    wpT_d = nc.dram_tensor("wpT", [128, FPC // 128, DIM], bf16, kind="ExternalInput")
    mask_d = nc.dram_tensor("mask01", [128, 128], bf16, kind="ExternalInput")
    ident_d = nc.dram_tensor("ident", [128, 128], bf16, kind="ExternalInput")
    out_d = nc.dram_tensor("out", [N, DIM], bf16, kind="ExternalOutput")

    NB = N // 128      # 16 blocks of 128 along sequence
    KC = DIM // 128    # 8 contraction chunks

    with tile.TileContext(nc) as tc, ExitStack() as ctx:
        sing = ctx.enter_context(tc.tile_pool(name="sing", bufs=1))
        pspool = ctx.enter_context(tc.tile_pool(name="pspool", bufs=2, space="PSUM"))
        o2pool = ctx.enter_context(tc.tile_pool(name="o2pool", bufs=2, space="PSUM"))
        ptpool = ctx.enter_context(tc.tile_pool(name="ptpool", bufs=4))
        rcpool = ctx.enter_context(tc.tile_pool(name="rcpool", bufs=4))
        ospool = ctx.enter_context(tc.tile_pool(name="ospool", bufs=4))
        outpool = ctx.enter_context(tc.tile_pool(name="outpool", bufs=3))

        if loop_k is not None:
            ctx.enter_context(tc.For_i(
                0, loop_k, 1,
                hint_engines=(
                    mybir.EngineType.PE, mybir.EngineType.Activation,
                    mybir.EngineType.DVE, mybir.EngineType.SP,
                ),
            ))

        # One table set serves every ACT use (Exp, Ln, Copy): preload it so
        # the automatic pass inserts no exp<->ln table swaps (~1.3us each).
        nc.scalar.add_instruction(mybir.InstLoadActFuncSet(
            name=nc.get_next_instruction_name(), ins=[], outs=[],
            act_func_set_id=6,  # natural_log_exp_and_others
        ))

        xTs = sing.tile([128, KC, N], bf16)
        wqTs = sing.tile([128, KC, FPC], bf16)
        wkTs = sing.tile([128, KC, FPC], bf16)
        wvTs = sing.tile([128, KC, FPC], bf16)
        wpTs = sing.tile([128, 2, DIM], bf16)
        qTs = sing.tile([128, 2, N], bf16)
        kTs = sing.tile([128, 2, N], bf16)
        vTs = sing.tile([128, 2, N], bf16)
        # v2: per (l-block j, head h) a contiguous 128-col weight slot:
        # even h -> [V_h | ones], odd h -> [ones | V_h]  (so O lands on
        # partitions [64*(h%2), +64) and the row-sums on the other half)
        v2 = sing.tile([128, NB, HPC, 128], bf16)
        onorm = sing.tile([128, 2, N], bf16)
        maskS = sing.tile([128, 128], bf16)
        identS = sing.tile([128, 128], bf16)

        # ---- input DMAs (first-needed first; wq per-chunk so the first
        # projection matmuls pipeline behind the x chunk arrivals) ----
        nc.sync.dma_start(out=wqTs[:, 0, :], in_=wqT_d[:, 0, :])
        for a in range(KC):
            nc.sync.dma_start(out=xTs[:, a, :], in_=xT_d[:, a, :])
            if a + 1 < KC:
                nc.sync.dma_start(out=wqTs[:, a + 1, :], in_=wqT_d[:, a + 1, :])
        nc.sync.dma_start(out=wkTs, in_=wkT_d[:])
        nc.sync.dma_start(out=wvTs, in_=wvT_d[:])
        nc.sync.dma_start(out=identS, in_=ident_d[:, :])
        nc.sync.dma_start(out=maskS, in_=mask_d[:, :])
        nc.sync.dma_start(out=wpTs, in_=wpT_d[:])

        for h in range(HPC):
            ones_cols = slice(64, 128) if h % 2 == 0 else slice(0, 64)
            nc.vector.memset(v2[:, :, h, ones_cols], 1.0)

        # ---- Q/K/V projections (weight-stationary, transposed outputs) ----
        def proj_block(wt, dst, t):
            for cc in range(2):
                ps = pspool.tile([128, 1024], f32, tag="ps", name=f"qkv_ps")
                for half in range(2):
                    n0 = 1024 * cc + 512 * half
                    for k in range(KC):
                        nc.tensor.matmul(
                            ps[:, 512 * half:512 * half + 512],
                            lhsT=wt[:, k, 128 * t:128 * (t + 1)],
                            rhs=xTs[:, k, n0:n0 + 512],
                            start=(k == 0), stop=(k == KC - 1),
                        )
                nc.vector.tensor_copy(
                    out=dst[:, t, 1024 * cc:1024 * (cc + 1)], in_=ps[:, :]
                )

        def v_transpose_block(t):
            # vTs[:, t, :] rows are d-dims of heads (2t, 2t+1); transpose each
            # 128x128 l-block back to [l, d] and scatter into the two heads'
            # v2 slots ([V|ones] / [ones|V]).
            vj_all = v2[:, :, :, :]
            part_d = list(vj_all.ap)[0]
            for j in range(NB):
                vt_ps = pspool.tile([128, 128], bf16, tag="ps", name="vt_ps")
                nc.tensor.transpose(
                    vt_ps[:, :], vTs[:, t, 128 * j:128 * (j + 1)], identS[:, :]
                )
                # dst: head 2t cols 0:64 and head 2t+1 cols 64:128
                dst = bass.AP(
                    tensor=vj_all.tensor,
                    offset=vj_all.offset + j * HPC * 128 + 256 * t,
                    ap=[[part_d[0], part_d[1]], [192, 2], [1, 64]],
                )
                nc.vector.tensor_copy(out=dst, in_=vt_ps[:, :])

        # ---- attention for head pair t, query half q (S^T -> exp -> O'') ----
        def attn_half(t, q, o2, after_j=None):
            nlo, nhi = 1024 * q, 1024 * (q + 1)
            for j in range(8 * (q + 1)):
                a0 = 128 * j
                lo = max(a0, nlo)
                pieces = _split512(lo, nhi)
                st = {par: pspool.tile([128, 1024], f32, tag="ps", name="st")
                      for par in range(2)}
                # even/odd adjacent per piece: K=64 at base partitions 0/64
                for p0, p1 in pieces:
                    for par in range(2):
                        r = 64 * par
                        nc.tensor.matmul(
                            st[par][:, p0 - nlo:p1 - nlo],
                            lhsT=kTs[r:r + 64, t, a0:a0 + 128],
                            rhs=qTs[r:r + 64, t, p0:p1],
                            start=True, stop=True,
                        )
                pt = {}
                for par in range(2):
                    pt[par] = ptpool.tile([128, 1024], bf16, tag="pt", name="pt")
                    nc.scalar.activation(
                        out=pt[par][:, lo - nlo:nhi - nlo],
                        in_=st[par][:, lo - nlo:nhi - nlo],
                        func=Exp, scale=SCALE,
                    )
                    if lo == a0:  # this chunk starts at the diagonal block
                        nc.vector.tensor_mul(
                            pt[par][:, a0 - nlo:a0 - nlo + 128],
                            pt[par][:, a0 - nlo:a0 - nlo + 128],
                            maskS,
                        )
                for par in range(2):
                    hh = 2 * t + par
                    vap = v2[:, j, hh, :]
                    for p0, p1 in pieces:
                        bank = p0 // 512
                        nc.tensor.matmul(
                            o2[par][:, p0 - nlo:p1 - nlo],
                            lhsT=vap,
                            rhs=pt[par][:, p0 - nlo:p1 - nlo],
                            start=(j == 0),
                            stop=(j == min(4 * bank + 3, 8 * (q + 1) - 1)),
                        )
                if after_j and j in after_j:
                    for fn in after_j[j]:
                        fn()

        def new_o2(t, q):
            return {par: o2pool.tile([128, 1024], f32, tag="o2",
                                     name=f"o2_{t}_{q}_{par}")
                    for par in range(2)}

        # Staged softmax normalize. release_half frees o2 fast: ACT takes
        # ln(row-sums) from PSUM while DVE copies the O rows to SBUF staging.
        # finish_half computes 1/s = exp(-ln s), DMA-shuffles it onto O's
        # partitions and multiplies into onorm -- off the o2-ring path.
        def release_half(t, q, o2):
            staged = {}
            for par in range(2):
                r = 64 * par
                sb = 64 - r
                rc = rcpool.tile([128, 1024], f32, tag="rc", name="rc")
                osg = ospool.tile([128, 1024], f32, tag="os", name="osg")
                nc.scalar.activation(
                    out=rc[sb:sb + 64, :], in_=o2[par][sb:sb + 64, :], func=Ln
                )
                nc.vector.tensor_copy(
                    out=osg[r:r + 64, :], in_=o2[par][r:r + 64, :]
                )
                staged[par] = (rc, osg)
            return staged

        def finish_half(t, q, staged):
            nlo = 1024 * q
            for par in range(2):
                r = 64 * par
                sb = 64 - r
                rc, osg = staged[par]
                nc.scalar.activation(
                    out=rc[sb:sb + 64, :], in_=rc[sb:sb + 64, :],
                    func=Exp, scale=-1.0,
                )
                nc.sync.dma_start(out=rc[r:r + 64, :], in_=rc[sb:sb + 64, :])
                nc.vector.tensor_mul(
                    out=onorm[r:r + 64, t, nlo:nlo + 1024],
                    in0=osg[r:r + 64, :], in1=rc[r:r + 64, :],
                )

        # ---- output projection: out[n_blk, e] = sum_pair O_pair.T @ WpT_pair ----
        def proj_out(nb):
            po = pspool.tile([128, 1024], f32, tag="ps", name="po")
            for half in range(2):
                for p in range(2):
                    nc.tensor.matmul(
                        po[:, 512 * half:512 * half + 512],
                        lhsT=onorm[:, p, 128 * nb:128 * (nb + 1)],
                        rhs=wpTs[:, p, 512 * half:512 * half + 512],
                        start=(p == 0), stop=(p == 1),
                    )
            ostage = outpool.tile([128, 1024], bf16, tag="ostage", name="ostage")
            if nb % 2 == 0:
                nc.vector.tensor_copy(out=ostage, in_=po)
            else:
                nc.scalar.copy(out=ostage, in_=po)
            nc.sync.dma_start(out=out_d[128 * nb:128 * (nb + 1), :], in_=ostage)

        # ---- schedule ----
        proj_block(wqTs, qTs, 0)
        proj_block(wkTs, kTs, 0)
        proj_block(wvTs, vTs, 0)
        v_transpose_block(0)

        o2 = new_o2(0, 0)
        attn_half(0, 0, o2)
        s00 = release_half(0, 0, o2)
        proj_block(wqTs, qTs, 1)          # boundary filler: independent PE work
        finish_half(0, 0, s00)

        o2 = new_o2(0, 1)
        attn_half(0, 1, o2)
        s01 = release_half(0, 1, o2)
        proj_block(wkTs, kTs, 1)          # filler
        finish_half(0, 1, s01)
        proj_block(wvTs, vTs, 1)
        v_transpose_block(1)

        o2 = new_o2(1, 0)
        attn_half(1, 0, o2)
        s10 = release_half(1, 0, o2)
        finish_half(1, 0, s10)

        # attention (1,1) starts immediately (o2 ring freed by the staged
        # release); once onorm rows 0-1023 are complete (a few j's in),
        # output-projection blocks 0-7 drop into the attention stream.
        o2 = new_o2(1, 1)
        s11 = {}
        def rel11():
            s11.update(release_half(1, 1, o2))
        attn_half(1, 1, o2, after_j={
            2: [lambda nb=nb: proj_out(nb) for nb in range(0, 4)],
            5: [lambda nb=nb: proj_out(nb) for nb in range(4, 8)],
        })
        rel11()
        finish_half(1, 1, s11)
        for nb in range(8, NB):
            proj_out(nb)

    nc.finalize()
    return nc


def _get_nc():
    global _built
    if _built is None:
        _built = _build()
    return _built


def _chunk128(a, kc):
    """[kc*128, m] -> [128, kc, m] so each partition row is contiguous."""
    m = a.shape[1]
    return np.ascontiguousarray(
        a.reshape(kc, 128, m).transpose(1, 0, 2)
    )


def make_in_maps(x, Wq, Wk, Wv, Wp):
    # 0 where key>query (strictly-lower in [l, n] coords), else 1
    mask = np.where(
        np.arange(128)[:, None] > np.arange(128)[None, :], 0.0, 1.0
    ).astype(_BF)
    ident = np.eye(128, dtype=np.float32).astype(_BF)
    in_maps = []
    for c in range(NCORES):
        b, g = c // HPC, c % HPC
        rows = slice(FPC * g, FPC * (g + 1))
        in_maps.append({
            "xT": _chunk128(np.ascontiguousarray(x[b].T).astype(_BF), DIM // 128),
            "wqT": _chunk128(np.ascontiguousarray(Wq[rows, :].T).astype(_BF), DIM // 128),
            "wkT": _chunk128(np.ascontiguousarray(Wk[rows, :].T).astype(_BF), DIM // 128),
            "wvT": _chunk128(np.ascontiguousarray(Wv[rows, :].T).astype(_BF), DIM // 128),
            "wpT": _chunk128(np.ascontiguousarray(Wp[:, rows].T).astype(_BF), FPC // 128),
            "mask01": mask,
            "ident": ident,
        })
    return in_maps


def run_sharded(x, Wq, Wk, Wv, Wp, bp, trace=False, **spmd_kwargs):
    from concourse.bass_utils import run_bass_kernel_spmd

    nc = _get_nc()
    in_maps = make_in_maps(x, Wq, Wk, Wv, Wp)
    res = run_bass_kernel_spmd(
        nc, in_maps, core_ids=list(range(NCORES)), trace=trace, **spmd_kwargs
    )
    parts = [r["out"] for r in res.results]
    out = np.zeros((B, N, DIM), np.float32)
    for b in range(B):
        acc = np.zeros((N, DIM), np.float32)
        for g in range(HPC):
            acc += parts[b * HPC + g].astype(np.float32)
        out[b] = acc + bp.astype(np.float32)[None, :]
    return out, res


def kernel(x, y, Wq, Wk, Wv, Wp, bp):
    x = np.asarray(x, np.float32)
    out, _ = run_sharded(
        x,
        np.asarray(Wq, np.float32), np.asarray(Wk, np.float32),
        np.asarray(Wv, np.float32), np.asarray(Wp, np.float32),
        np.asarray(bp, np.float32),
    )
    return out


# revision 12
# speedup vs baseline: 1.2730x; 1.0581x over previous
"""Causal self-attention block (nn_CrossAttention) on 8 TRN2 NeuronCores.

Sharding: data-parallel over batch (B=2 -> 2 groups of 4 cores), tensor-parallel
over heads within a group (16 heads -> 4 heads/core, splitting Wq/Wk/Wv rows and
Wp columns). Each core computes a full [N, DIM] partial of the output projection
for its 4 heads; the host sums the 4 partials per batch and adds the bias.

Device-side layout ("transposed world", everything feature-major):
  xT   [C=1024, N=2048]    QT/KT/VT = W @ xT -> [d, n] with d on partitions
  V    = PE-transpose of VT blocks -> [l, d] per 128-block, packed per head
         with a 64-wide ones block ([V_h|ones] even heads, [ones|V_h] odd)
  S^T  = K_j @ Q^T chunks  -> [l, n] in PSUM (l = key block on partitions)
  P^T  = exp(SCALE*S^T) -> SBUF bf16, causal-masked by a 0/1 multiply
  O''  = [V_j|ones].T @ P^T accumulated in PSUM: O rows + row-sum rows
  out  = (O/s).T-pair @ WpT -> [n, e] partial, bf16 to DRAM

No max-subtraction is needed in the softmax (logits*scale max ~8).

Softmax normalization is division-free and staged so the PSUM accumulator
ring never waits on it: o2 is released by a fast pair (DVE copies the O
rows to SBUF staging || ACT takes ln of the row-sums straight from PSUM);
1/s = exp(-ln s) (ACT; Exp+Ln+Copy share table set 6,
natural_log_exp_and_others, preloaded once so the table-load pass emits
no swaps), a DMA shuffles 1/s onto O's partitions, and DVE multiplies
into onorm -- all off the o2-ring critical path.

Schedule: t=1 QKV/V-transpose blocks fill the (t=0) normalize boundaries;
attention (1,1) starts right after (1,0) with only the staged release in
between; output-projection row blocks 0-7 are injected into attention
(1,1) once onorm rows 0-1023 exist, overlapping the final attention phase
and spreading the (bf16) output DMA.
"""

import numpy as np
import ml_dtypes

B = 2
N = 2048
DIM = 1024
H = 16
D = 64
SCALE = D ** -0.5
NCORES = 8
HPC = 4          # heads per core
FPC = HPC * D    # feature rows per core (256)

_BF = ml_dtypes.bfloat16

_built = None


def _split512(lo, hi):
    """Split [lo, hi) at multiples of 512 (PSUM bank boundaries)."""
    out = []
    p = lo
    while p < hi:
        q = min(hi, (p // 512 + 1) * 512)
        out.append((p, q))
        p = q
    return out


def _build(loop_k=None):
    """Build the (SPMD, data-only-sharded) Bass program. Same NEFF on all cores.

    loop_k: if set, wrap the whole compute body in a hardware For-loop that
    repeats it loop_k times (used only for timing-by-differencing in bench.py;
    the graded path uses loop_k=None).
    """
    import concourse.bass as bass
    import concourse.mybir as mybir
    import concourse.tile as tile
    from concourse import bacc
    from contextlib import ExitStack

    bf16 = mybir.dt.bfloat16
    f32 = mybir.dt.float32
    Exp = mybir.ActivationFunctionType.Exp
    Ln = mybir.ActivationFunctionType.Ln

    nc = bacc.Bacc()
    # Host pre-lays-out weights so every DMA row is 4KB-contiguous per
    # partition (512B packets from the "(a p) d" rearrange are ~4x slower).
    xT_d = nc.dram_tensor("xT", [128, DIM // 128, N], bf16, kind="ExternalInput")
    wqT_d = nc.dram_tensor("wqT", [128, DIM // 128, FPC], bf16, kind="ExternalInput")
    wkT_d = nc.dram_tensor("wkT", [128, DIM // 128, FPC], bf16, kind="ExternalInput")
    wvT_d = nc.dram_tensor("wvT", [128, DIM // 128, FPC], bf16, kind="ExternalInput")
    wpT_d = nc.dram_tensor("wpT", [128, FPC // 128, DIM], bf16, kind="ExternalInput")
    mask_d = nc.dram_tensor("mask01", [128, 128], bf16, kind="ExternalInput")
    ident_d = nc.dram_tensor("ident", [128, 128], bf16, kind="ExternalInput")
    out_d = nc.dram_tensor("out", [N, DIM], bf16, kind="ExternalOutput")

    NB = N // 128      # 16 blocks of 128 along sequence
    KC = DIM // 128    # 8 contraction chunks

    with tile.TileContext(nc) as tc, ExitStack() as ctx:
        sing = ctx.enter_context(tc.tile_pool(name="sing", bufs=1))
        pspool = ctx.enter_context(tc.tile_pool(name="pspool", bufs=2, space="PSUM"))
        o2pool = ctx.enter_context(tc.tile_pool(name="o2pool", bufs=2, space="PSUM"))
        ptpool = ctx.enter_context(tc.tile_pool(name="ptpool", bufs=4))
        rcpool = ctx.enter_context(tc.tile_pool(name="rcpool", bufs=4))
        ospool = ctx.enter_context(tc.tile_pool(name="ospool", bufs=4))
        outpool = ctx.enter_context(tc.tile_pool(name="outpool", bufs=3))

        if loop_k is not None:
            ctx.enter_context(tc.For_i(
                0, loop_k, 1,
                hint_engines=(
                    mybir.EngineType.PE, mybir.EngineType.Activation,
                    mybir.EngineType.DVE, mybir.EngineType.SP,
                ),
            ))

        # One table set serves every ACT use (Exp, Ln, Copy): preload it so
        # the automatic pass inserts no exp<->ln table swaps (~1.3us each).
        nc.scalar.add_instruction(mybir.InstLoadActFuncSet(
            name=nc.get_next_instruction_name(), ins=[], outs=[],
            act_func_set_id=6,  # natural_log_exp_and_others
        ))

        xTs = sing.tile([128, KC, N], bf16)
        wqTs = sing.tile([128, KC, FPC], bf16)
        wkTs = sing.tile([128, KC, FPC], bf16)
        wvTs = sing.tile([128, KC, FPC], bf16)
        wpTs = sing.tile([128, 2, DIM], bf16)
        qTs = sing.tile([128, 2, N], bf16)
        kTs = sing.tile([128, 2, N], bf16)
        vTs = sing.tile([128, 2, N], bf16)
        # v2: per (l-block j, head h) a contiguous 128-col weight slot:
        # even h -> [V_h | ones], odd h -> [ones | V_h]  (so O lands on
        # partitions [64*(h%2), +64) and the row-sums on the other half)
        v2 = sing.tile([128, NB, HPC, 128], bf16)
        onorm = sing.tile([128, 2, N], bf16)
        maskS = sing.tile([128, 128], bf16)
        identS = sing.tile([128, 128], bf16)

        # ---- input DMAs (first-needed first; wq per-chunk so the first
        # projection matmuls pipeline behind the x chunk arrivals) ----
        nc.sync.dma_start(out=wqTs[:, 0, :], in_=wqT_d[:, 0, :])
        for a in range(KC):
            nc.sync.dma_start(out=xTs[:, a, :], in_=xT_d[:, a, :])
            if a + 1 < KC:
                nc.sync.dma_start(out=wqTs[:, a + 1, :], in_=wqT_d[:, a + 1, :])
        nc.sync.dma_start(out=wkTs, in_=wkT_d[:])
        nc.sync.dma_start(out=wvTs, in_=wvT_d[:])
        nc.sync.dma_start(out=identS, in_=ident_d[:, :])
        nc.sync.dma_start(out=maskS, in_=mask_d[:, :])
        nc.sync.dma_start(out=wpTs, in_=wpT_d[:])

        for h in range(HPC):
            ones_cols = slice(64, 128) if h % 2 == 0 else slice(0, 64)
            nc.vector.memset(v2[:, :, h, ones_cols], 1.0)

        # ---- Q/K/V projections (weight-stationary, transposed outputs) ----
        def qkv_cc(wt, dst, t, cc):
            ps = pspool.tile([128, 1024], f32, tag="ps", name=f"qkv_ps")
            for half in range(2):
                n0 = 1024 * cc + 512 * half
                for k in range(KC):
                    nc.tensor.matmul(
                        ps[:, 512 * half:512 * half + 512],
                        lhsT=wt[:, k, 128 * t:128 * (t + 1)],
                        rhs=xTs[:, k, n0:n0 + 512],
                        start=(k == 0), stop=(k == KC - 1),
                    )
            nc.vector.tensor_copy(
                out=dst[:, t, 1024 * cc:1024 * (cc + 1)], in_=ps[:, :]
            )

        def proj_block(wt, dst, t):
            for cc in range(2):
                qkv_cc(wt, dst, t, cc)

        # Dependency-free PE activity: the HAM clock gate demotes the PE to
        # 1.2 GHz after ~3.4us windows with idle time; a burst of standalone
        # LDWEIGHTS (~0.1us each, no consumers) keeps it at 2.4 GHz through
        # stretches where attention is ACT(exp)-bound and no real matmul is
        # ready.
        def warm(n):
            for _ in range(n):
                nc.tensor.ldweights(maskS[:, :])

        def v_transpose_block(t):
            # vTs[:, t, :] rows are d-dims of heads (2t, 2t+1); transpose each
            # 128x128 l-block back to [l, d] and scatter into the two heads'
            # v2 slots ([V|ones] / [ones|V]).
            vj_all = v2[:, :, :, :]
            part_d = list(vj_all.ap)[0]
            for j in range(NB):
                vt_ps = pspool.tile([128, 128], bf16, tag="ps", name="vt_ps")
                nc.tensor.transpose(
                    vt_ps[:, :], vTs[:, t, 128 * j:128 * (j + 1)], identS[:, :]
                )
                # dst: head 2t cols 0:64 and head 2t+1 cols 64:128
                dst = bass.AP(
                    tensor=vj_all.tensor,
                    offset=vj_all.offset + j * HPC * 128 + 256 * t,
                    ap=[[part_d[0], part_d[1]], [192, 2], [1, 64]],
                )
                nc.vector.tensor_copy(out=dst, in_=vt_ps[:, :])

        # ---- attention for head pair t, query half q (S^T -> exp -> O'') ----
        # Software-pipelined one key-block ahead: the PE stream is
        # ... S(j+1) O(j) S(j+2) O(j+1) ..., so each O finds its exp'd P
        # ready and the PE never idles waiting on ACT — sub-us PE bubbles
        # would demote the HAM clock gate from 2.4 to 1.2 GHz for >=3.4us.
        def attn_half(t, q, o2, after_j=None):
            nlo, nhi = 1024 * q, 1024 * (q + 1)
            NJ = 8 * (q + 1)
            pts = {}

            def emit_S(j):
                a0 = 128 * j
                lo = max(a0, nlo)
                pieces = _split512(lo, nhi)
                st = {par: pspool.tile([128, 1024], f32, tag="ps", name="st")
                      for par in range(2)}
                # even/odd adjacent per piece: K=64 at base partitions 0/64
                for p0, p1 in pieces:
                    for par in range(2):
                        r = 64 * par
                        nc.tensor.matmul(
                            st[par][:, p0 - nlo:p1 - nlo],
                            lhsT=kTs[r:r + 64, t, a0:a0 + 128],
                            rhs=qTs[r:r + 64, t, p0:p1],
                            start=True, stop=True,
                        )
                pt = {}
                for par in range(2):
                    pt[par] = ptpool.tile([128, 1024], bf16, tag="pt", name="pt")
                    nc.scalar.activation(
                        out=pt[par][:, lo - nlo:nhi - nlo],
                        in_=st[par][:, lo - nlo:nhi - nlo],
                        func=Exp, scale=SCALE,
                    )
                    if lo == a0:  # this chunk starts at the diagonal block
                        nc.vector.tensor_mul(
                            pt[par][:, a0 - nlo:a0 - nlo + 128],
                            pt[par][:, a0 - nlo:a0 - nlo + 128],
                            maskS,
                        )
                pts[j] = pt

            def emit_O(j):
                a0 = 128 * j
                lo = max(a0, nlo)
                pieces = _split512(lo, nhi)
                pt = pts.pop(j)
                for par in range(2):
                    hh = 2 * t + par
                    vap = v2[:, j, hh, :]
                    for p0, p1 in pieces:
                        bank = p0 // 512
                        nc.tensor.matmul(
                            o2[par][:, p0 - nlo:p1 - nlo],
                            lhsT=vap,
                            rhs=pt[par][:, p0 - nlo:p1 - nlo],
                            start=(j == 0),
                            stop=(j == min(4 * bank + 3, NJ - 1)),
                        )

            emit_S(0)
            for j in range(NJ):
                if j + 1 < NJ:
                    emit_S(j + 1)
                emit_O(j)
                if after_j and j in after_j:
                    for fn in after_j[j]:
                        fn()

        def new_o2(t, q):
            return {par: o2pool.tile([128, 1024], f32, tag="o2",
                                     name=f"o2_{t}_{q}_{par}")
                    for par in range(2)}

        # Staged softmax normalize. release_half frees o2 fast: ACT takes
        # ln(row-sums) from PSUM while DVE copies the O rows to SBUF staging.
        # finish_half computes 1/s = exp(-ln s), DMA-shuffles it onto O's
        # partitions and multiplies into onorm -- off the o2-ring path.
        def release_half(t, q, o2):
            staged = {}
            for par in range(2):
                r = 64 * par
                sb = 64 - r
                rc = rcpool.tile([128, 1024], f32, tag="rc", name="rc")
                osg = ospool.tile([128, 1024], f32, tag="os", name="osg")
                nc.scalar.activation(
                    out=rc[sb:sb + 64, :], in_=o2[par][sb:sb + 64, :], func=Ln
                )
                nc.vector.tensor_copy(
                    out=osg[r:r + 64, :], in_=o2[par][r:r + 64, :]
                )
                staged[par] = (rc, osg)
            return staged

        def finish_half(t, q, staged):
            nlo = 1024 * q
            for par in range(2):
                r = 64 * par
                sb = 64 - r
                rc, osg = staged[par]
                nc.scalar.activation(
                    out=rc[sb:sb + 64, :], in_=rc[sb:sb + 64, :],
                    func=Exp, scale=-1.0,
                )
                nc.sync.dma_start(out=rc[r:r + 64, :], in_=rc[sb:sb + 64, :])
                nc.vector.tensor_mul(
                    out=onorm[r:r + 64, t, nlo:nlo + 1024],
                    in0=osg[r:r + 64, :], in1=rc[r:r + 64, :],
                )

        # ---- output projection: out[n_blk, e] = sum_pair O_pair.T @ WpT_pair ----
        def proj_out(nb):
            po = pspool.tile([128, 1024], f32, tag="ps", name="po")
            for half in range(2):
                for p in range(2):
                    nc.tensor.matmul(
                        po[:, 512 * half:512 * half + 512],
                        lhsT=onorm[:, p, 128 * nb:128 * (nb + 1)],
                        rhs=wpTs[:, p, 512 * half:512 * half + 512],
                        start=(p == 0), stop=(p == 1),
                    )
            ostage = outpool.tile([128, 1024], bf16, tag="ostage", name="ostage")
            if nb % 2 == 0:
                nc.vector.tensor_copy(out=ostage, in_=po)
            else:
                nc.scalar.copy(out=ostage, in_=po)
            nc.sync.dma_start(out=out_d[128 * nb:128 * (nb + 1), :], in_=ostage)

        # ---- schedule: pair ACT-bound attention with PE-heavy fill at fine
        # grain so the PE never idles (and the HAM clock gate stays warm) ----
        proj_block(wqTs, qTs, 0)
        proj_block(wkTs, kTs, 0)
        proj_block(wvTs, vTs, 0)
        v_transpose_block(0)

        # attention (0,0): fill with the t=1 Q projection
        o2 = new_o2(0, 0)
        attn_half(0, 0, o2, after_j={
            2: [lambda: qkv_cc(wqTs, qTs, 1, 0)],
            5: [lambda: qkv_cc(wqTs, qTs, 1, 1)],
        })
        s00 = release_half(0, 0, o2)
        finish_half(0, 0, s00)

        # attention (0,1): fill with the t=1 K/V projections + V transpose
        o2 = new_o2(0, 1)
        attn_half(0, 1, o2, after_j={
            1: [lambda: qkv_cc(wkTs, kTs, 1, 0)],
            4: [lambda: qkv_cc(wkTs, kTs, 1, 1)],
            7: [lambda: qkv_cc(wvTs, vTs, 1, 0)],
            9: [lambda: qkv_cc(wvTs, vTs, 1, 1)],
            11: [lambda: v_transpose_block(1)],
        })
        s01 = release_half(0, 1, o2)
        finish_half(0, 1, s01)

        # attention (1,0): no independent real work exists (the output
        # projection needs this block's own normalize) -- keep the PE warm
        o2 = new_o2(1, 0)
        attn_half(1, 0, o2, after_j={j: [lambda: warm(6)] for j in range(8)})
        s10 = release_half(1, 0, o2)
        finish_half(1, 0, s10)

        # attention (1,1): fill with output-projection blocks 0-7 (their
        # onorm rows 0-1023 are complete) plus warm-keepers
        o2 = new_o2(1, 1)
        attn_half(1, 1, o2, after_j={
            2: [lambda: proj_out(0), lambda: proj_out(1)],
            5: [lambda: proj_out(2), lambda: proj_out(3)],
            8: [lambda: proj_out(4), lambda: proj_out(5)],
            11: [lambda: proj_out(6), lambda: proj_out(7)],
            **{j: [lambda: warm(3)] for j in (1, 3, 4, 6, 7, 9, 10, 12, 13, 14)},
        })
        s11 = release_half(1, 1, o2)
        warm(24)      # bridge the last normalize so proj 8-15 runs at 2.4 GHz
        finish_half(1, 1, s11)
        for nb in range(8, NB):
            proj_out(nb)

    nc.finalize()
    return nc


def _get_nc():
    global _built
    if _built is None:
        _built = _build()
    return _built


def _chunk128(a, kc):
    """[kc*128, m] -> [128, kc, m] so each partition row is contiguous."""
    m = a.shape[1]
    return np.ascontiguousarray(
        a.reshape(kc, 128, m).transpose(1, 0, 2)
    )


def make_in_maps(x, Wq, Wk, Wv, Wp):
    # 0 where key>query (strictly-lower in [l, n] coords), else 1
    mask = np.where(
        np.arange(128)[:, None] > np.arange(128)[None, :], 0.0, 1.0
    ).astype(_BF)
    ident = np.eye(128, dtype=np.float32).astype(_BF)
    in_maps = []
    for c in range(NCORES):
        b, g = c // HPC, c % HPC
        rows = slice(FPC * g, FPC * (g + 1))
        in_maps.append({
            "xT": _chunk128(np.ascontiguousarray(x[b].T).astype(_BF), DIM // 128),
            "wqT": _chunk128(np.ascontiguousarray(Wq[rows, :].T).astype(_BF), DIM // 128),
            "wkT": _chunk128(np.ascontiguousarray(Wk[rows, :].T).astype(_BF), DIM // 128),
            "wvT": _chunk128(np.ascontiguousarray(Wv[rows, :].T).astype(_BF), DIM // 128),
            "wpT": _chunk128(np.ascontiguousarray(Wp[:, rows].T).astype(_BF), FPC // 128),
            "mask01": mask,
            "ident": ident,
        })
    return in_maps


def run_sharded(x, Wq, Wk, Wv, Wp, bp, trace=False, **spmd_kwargs):
    from concourse.bass_utils import run_bass_kernel_spmd

    nc = _get_nc()
    in_maps = make_in_maps(x, Wq, Wk, Wv, Wp)
    res = run_bass_kernel_spmd(
        nc, in_maps, core_ids=list(range(NCORES)), trace=trace, **spmd_kwargs
    )
    parts = [r["out"] for r in res.results]
    out = np.zeros((B, N, DIM), np.float32)
    for b in range(B):
        acc = np.zeros((N, DIM), np.float32)
        for g in range(HPC):
            acc += parts[b * HPC + g].astype(np.float32)
        out[b] = acc + bp.astype(np.float32)[None, :]
    return out, res


def kernel(x, y, Wq, Wk, Wv, Wp, bp):
    x = np.asarray(x, np.float32)
    out, _ = run_sharded(
        x,
        np.asarray(Wq, np.float32), np.asarray(Wk, np.float32),
        np.asarray(Wv, np.float32), np.asarray(Wp, np.float32),
        np.asarray(bp, np.float32),
    )
    return out
